# revision 22
# baseline (speedup 1.0000x reference)
"""Trainium2 Bass kernel for CTC loss (K.ctc_batch_cost semantics).

Problem (hardcoded): B=1024, T=256, C=128, L=32, blank=C-1, S=2L+1=65.
Sharding: pure data parallel, 128 examples per core across 8 cores.

Device algorithm (per core) — prescaled linear-domain state sweep:

    alpha_hat[t,s] = (y[t-1,s] + alpha_hat[t-1,s]) * E[t,s]
    y[t,s]   = alpha_hat[t,s-1] + m2[s]*alpha_hat[t,s-2]
    E[t,s]   = (p[b,t,ext[s]] + eps) * K_w          (constant K per window)

Even (blank) states all share E row 0, so only 33 emission rows per
example (blank + 32 labels) are gathered, and even states need no
scalar_tensor_tensor (m2 is 0 there).  The per-window constants K_w
keep alpha_hat inside fp32 range with no cross-window re-anchoring, and
are accounted exactly on the host: loss = TW*sum(ln K_w) - ln(fin).

Pipeline per 64-step window w, group g (32 examples):
  DMA y_pred [t-part, e, c] -> gpsimd ap_gather of 33 class-columns per
  example -> PE transpose per state (strided input, 4 groups fused) ->
  ACT copy psum->E with scale=K_w, bias=K_w*eps.
Sweeps: windows 0-2 run on DVE (hidden under the DMA stream), window 3
on Pool (faster per-op) to minimize the tail after the last DMA.
"""

import numpy as np

EPS = 1e-7
B_TOT, T, C, L = 1024, 256, 128, 32
NCORES = 8
B = B_TOT // NCORES          # 128 examples per core
S = 2 * L + 1                # 65
TW = 64                      # window size (time steps)
NWIN = T // TW               # 4
EG = 32                      # examples per DMA group
NGRP = B // EG               # 4 groups
NST = L + 1                  # 33 gathered rows: blank + odd states
NIDX = 16 * NST              # 528 gather indices per partition
GST = NIDX // 16 + 1         # 34: per-group idx-table cols, padded so each
                             # group's idx base stays 4-byte aligned
SER = T + 1                  # series cols per state (col 0 == t=-1)
KW = [64.0, 64.0, 64.0 * float(np.exp(0.25)), 64.0 * float(np.exp(0.25))]

_CACHE = {}


# ----------------------------------------------------------------------------
# host-side tables
# ----------------------------------------------------------------------------

def _host_tables(y_true):
    """Wrapped gather tables per core: [NCORES, 128, NGRP*NIDX/16] int16.

    pt tiles hold [(a t) part, 16 e, 128 c] — partition p = a*64 + t where
    a picks the 16-example subgroup.  Partitions 0..63 gather subgroup a=0
    (examples g*32+0..15), partitions 64..127 subgroup a=1.  Per subgroup:
    flat[j*16 + e] = e*C + cls(e, j); j=0 is blank, j>=1 label j-1.
    """
    lab = np.asarray(y_true).astype(np.int32)
    cls = np.full((B_TOT, NST), C - 1, np.int32)
    cls[:, 1:] = lab
    tables = np.zeros((NCORES, NGRP, 128, GST), np.int16)
    for core in range(NCORES):
        for g in range(NGRP):
            for a in range(2):
                b0 = core * B + g * EG + a * 16
                flat = (np.arange(16)[None, :] * C
                        + cls[b0:b0 + 16].T).reshape(-1)
                wrapped = flat.reshape(NIDX // 16, 16).T   # [16, NIDX/16]
                tables[core, g, a * 64:(a + 1) * 64, :NIDX // 16] = np.tile(
                    wrapped, (4, 1)).astype(np.int16)
    return np.ascontiguousarray(tables.transpose(0, 2, 1, 3).reshape(
        NCORES, 128, -1))


def _host_mask2(y_true):
    """m2[b, j] for odd state 2j-1: 1 if the s-2 skip is allowed. [B_TOT, NST]."""
    lab = np.asarray(y_true).astype(np.int32)
    m2 = np.zeros((B_TOT, NST), np.float32)
    m2[:, 2:] = (lab[:, 1:] != lab[:, :-1]).astype(np.float32)
    return m2


# ----------------------------------------------------------------------------
# device kernel
# ----------------------------------------------------------------------------

def _build_module():
    import concourse.bacc as bacc
    import concourse.mybir as mybir
    import concourse.tile as tile
    from concourse import library_config
    from concourse.tile_rust import add_dep_helper

    dt = mybir.dt
    AF = mybir.ActivationFunctionType
    OP = mybir.AluOpType

    nc = bacc.Bacc("TRN2", target_bir_lowering=False, debug=False,
                   enable_asserts=False, num_devices=NCORES)

    # y_pred pre-shuffled on host: [(a t), (w g e c)] so each (w, g) DMA
    # is one fully-contiguous 8KB-per-partition transfer on 128 partitions.
    yp = nc.dram_tensor("yp2", [128, NWIN * NGRP * 16 * C], dt.float32,
                        kind="ExternalInput")
    gtab = nc.dram_tensor("gtab", [128, NGRP * GST], dt.int16,
                          kind="ExternalInput")
    m2_in = nc.dram_tensor("m2", [B, NST], dt.float32, kind="ExternalInput")
    ident_in = nc.dram_tensor("ident", [64, 64], dt.float32,
                              kind="ExternalInput")
    fin_out = nc.dram_tensor("fin", [B, 1], dt.float32, kind="ExternalOutput")

    with tile.TileContext(nc) as tc:
        with (
            tc.tile_pool(name="const", bufs=1) as cpool,
            tc.tile_pool(name="pin", bufs=2) as ppool,
            tc.tile_pool(name="eg", bufs=2) as gpool,
            tc.tile_pool(name="ybuf", bufs=4) as ypool,
            tc.tile_pool(name="small", bufs=1) as spool,
            tc.tile_pool(name="tp", bufs=4, space="PSUM") as tpool,
        ):
            ident_sb = cpool.tile([64, 64], dt.float32, name="ident_sb")
            nc.sync.dma_start(ident_sb, ident_in[:, :])
            gtab_sb = cpool.tile([128, NGRP * GST], dt.int16,
                                 name="gtab_sb")
            nc.sync.dma_start(gtab_sb, gtab[:, :])
            m2_sb = cpool.tile([B, NST], dt.float32, name="m2_sb")
            nc.sync.dma_start(m2_sb, m2_in[:, :])

            lib_inst = nc.gpsimd.load_library(library_config.ap_gather)

            # alpha_hat series: [128, S, SER] fp32; col 0 = t=-1 (zeros)
            series = spool.tile([B, S * SER], dt.float32, name="series")
            ser_v = series.rearrange("p (s t) -> p s t", t=SER)
            nc.vector.memset(ser_v[:, :, 0], 0.0)

            zeros_f = spool.tile([B, TW], dt.float32, name="zeros_f")
            nc.vector.memset(zeros_f, 0.0)

            epsb = []
            for w in range(NWIN):
                if w > 0 and KW[w] == KW[w - 1]:
                    epsb.append(epsb[-1])
                    continue
                eb = spool.tile([128, 1], dt.float32, name=f"epsb{w}")
                nc.vector.memset(eb, KW[w] * EPS)
                epsb.append(eb)

            fin = spool.tile([B, 1], dt.float32, name="fin")

            # emission rows per window: [128 ex, NST*TW] fp32
            ecomb = []
            for w in range(NWIN):
                e_t = spool.tile([B, NST * TW], dt.float32, name=f"ecomb{w}")
                ecomb.append(e_t)

            def prep_window(w):
                """DMA + gather + reorder + transpose + scaled copy."""
                egath = gpool.tile([128, NGRP * NIDX], dt.float32, tag="eg",
                                   name=f"egath{w}")
                est = gpool.tile([64, NST * B], dt.float32, tag="est",
                                 name=f"est{w}")
                esv = est.rearrange("p (j g a e) -> p j g a e",
                                    g=NGRP, a=2, j=NST)
                for g in range(NGRP):
                    # [(a t) part, 16 e, 128 c]: a = 16-example subgroup
                    ptile = ppool.tile([128, 16 * C], dt.float32, tag="pt",
                                       name=f"pt{w}_{g}")
                    blk = (w * NGRP + g) * 16 * C
                    nc.sync.dma_start(ptile, yp[:, blk:blk + 16 * C])
                    gi = nc.gpsimd.ap_gather(
                        egath[:, g * NIDX:(g + 1) * NIDX], ptile,
                        gtab_sb[:, g * GST:g * GST + NIDX // 16],
                        channels=128, num_elems=16 * C, d=1, num_idxs=NIDX)
                    add_dep_helper(lib_inst.ins, gi.ins, sync=False,
                                   reason="library before gather")
                    # reorder both subgroup halves into example-major est
                    egv = egath[:, g * NIDX:(g + 1) * NIDX] \
                        .rearrange("p (j e) -> p j e", j=NST)
                    for a in range(2):
                        nc.scalar.activation(
                            esv[:, :, g, a, :],
                            egv[a * 64:(a + 1) * 64, :, :], AF.Copy)
                # PE transpose per state, ACT copy psum->E with
                # (x+eps)*K fused via scale/bias.
                ecv = ecomb[w]
                for j0 in range(0, NST, 4):
                    ns = min(4, NST - j0)
                    tp = tpool.tile([128, 4 * TW], dt.float32, tag="tp",
                                    name=f"tp{w}_{j0}")
                    for k in range(ns):
                        nc.tensor.transpose(
                            tp[:, k * TW:(k + 1) * TW],
                            est[:, (j0 + k) * B:(j0 + k + 1) * B], ident_sb)
                    nc.scalar.activation(
                        ecv[:, j0 * TW:(j0 + ns) * TW],
                        tp[:, 0:ns * TW], AF.Identity,
                        bias=epsb[w], scale=KW[w])

            def sweep_window(w, eng):
                """Run the s-sweep scans for window w on engine `eng`."""
                t0 = w * TW
                ecv = ecomb[w].rearrange("p (j t) -> p j t", t=TW)
                for s in range(S):
                    out_ap = ser_v[:, s, t0 + 1:t0 + 1 + TW]
                    if w == 0:
                        init = 1.0 if s <= 1 else 0.0
                    else:
                        init = ser_v[:, s, t0:t0 + 1]
                    if s == 0:
                        d0 = zeros_f
                    elif s % 2 == 0 or s == 1:
                        d0 = ser_v[:, s - 1, t0:t0 + TW]
                    else:
                        j = (s + 1) // 2
                        yb = ypool.tile([B, TW], dt.float32, tag="yb",
                                        name=f"yb{w}_{s}")
                        eng.scalar_tensor_tensor(
                            yb, ser_v[:, s - 2, t0:t0 + TW],
                            m2_sb[:, j:j + 1], ser_v[:, s - 1, t0:t0 + TW],
                            op0=OP.mult, op1=OP.add)
                        d0 = yb
                    ej = 0 if s % 2 == 0 else (s + 1) // 2
                    eng.tensor_tensor_scan(
                        out_ap, d0, ecv[:, ej, :], init,
                        op0=OP.add, op1=OP.mult)

            for w in range(NWIN):
                prep_window(w)
            for w in range(NWIN):
                sweep_window(w, nc.vector)

            nc.vector.tensor_add(fin, ser_v[:, S - 2, T:T + 1],
                                 ser_v[:, S - 1, T:T + 1])
            nc.sync.dma_start(fin_out[:, :], fin)

    nc.compile()
    return nc


def _get_module():
    if "nc" not in _CACHE:
        _CACHE["nc"] = _build_module()
    return _CACHE["nc"]


# ----------------------------------------------------------------------------
# entry point
# ----------------------------------------------------------------------------

def _feeds(y_true, y_pred):
    y_pred = np.asarray(y_pred, dtype=np.float32)
    tables = _host_tables(y_true)
    m2 = _host_mask2(y_true)
    ident = np.eye(64, dtype=np.float32)
    maps = []
    for core in range(NCORES):
        ypc = y_pred[core * B:(core + 1) * B]
        # [(a tl), (w g e c)]: partition p = a*64 + tl
        yp2 = np.ascontiguousarray(
            ypc.reshape(NGRP, 2, 16, NWIN, TW, C)
            .transpose(1, 4, 3, 0, 2, 5).reshape(128, -1))
        maps.append({
            "yp2": yp2,
            "gtab": tables[core],
            "m2": m2[core * B:(core + 1) * B],
            "ident": ident,
        })
    return maps


def _run(y_true, y_pred, trace=False):
    from concourse.bass_utils import run_bass_kernel_spmd
    nc = _get_module()
    return run_bass_kernel_spmd(nc, _feeds(y_true, y_pred),
                                core_ids=list(range(NCORES)), trace=trace)


def kernel(y_true, y_pred):
    res = _run(y_true, y_pred)
    lnk = TW * float(np.sum(np.log(KW)))
    out = np.zeros(B_TOT, np.float64)
    for i in range(NCORES):
        fin = res.results[i]["fin"].reshape(B).astype(np.float64)
        out[i * B:(i + 1) * B] = lnk - np.log(fin)
    return out.astype(np.float32)[:, None]


def profile_once(y_true, y_pred):
    res = _run(y_true, y_pred, trace=True)
    return res.exec_time_ns


if __name__ == "__main__":
    rng = np.random.default_rng(0)
    yt = rng.integers(0, 126, size=(B_TOT, L)).astype(np.int64)
    logits = rng.standard_normal((B_TOT, T, C)).astype(np.float32)
    ex = np.exp(logits - logits.max(-1, keepdims=True))
    ypred = (ex / ex.sum(-1, keepdims=True)).astype(np.float32)
    out = kernel(yt, ypred)
    print("out", out.shape, out[:4, 0])


# revision 24
# speedup vs baseline: 1.0165x; 1.0165x over previous
"""Trainium2 Bass kernel for CTC loss (K.ctc_batch_cost semantics).

Problem (hardcoded): B=1024, T=256, C=128, L=32, blank=C-1, S=2L+1=65.
Sharding: pure data parallel, 128 examples per core across 8 cores.

Device algorithm (per core) — prescaled linear-domain state sweep:

    alpha_hat[t,s] = (y[t-1,s] + alpha_hat[t-1,s]) * E[t,s]
    y[t,s]   = alpha_hat[t,s-1] + m2[s]*alpha_hat[t,s-2]
    E[t,s]   = (p[b,t,ext[s]] + eps) * K_w          (constant K per window)

Even (blank) states all share E row 0, so only 33 emission rows per
example (blank + 32 labels) are gathered, and even states need no
scalar_tensor_tensor (m2 is 0 there).  The per-window constants K_w
keep alpha_hat inside fp32 range with no cross-window re-anchoring, and
are accounted exactly on the host: loss = TW*sum(ln K_w) - ln(fin).

Pipeline per 64-step window w, group g (32 examples):
  DMA y_pred [t-part, e, c] -> gpsimd ap_gather of 33 class-columns per
  example -> PE transpose per state (strided input, 4 groups fused) ->
  ACT copy psum->E with scale=K_w, bias=K_w*eps.
Sweeps: windows 0-2 run on DVE (hidden under the DMA stream), window 3
on Pool (faster per-op) to minimize the tail after the last DMA.
"""

import numpy as np

EPS = 1e-7
B_TOT, T, C, L = 1024, 256, 128, 32
NCORES = 8
B = B_TOT // NCORES          # 128 examples per core
S = 2 * L + 1                # 65
TW = 64                      # window size (time steps)
NWIN = T // TW               # 4
EG = 32                      # examples per DMA group
NGRP = B // EG               # 4 groups
NST = L + 1                  # 33 gathered rows: blank + odd states
NIDX = 16 * NST              # 528 gather indices per partition
GST = NIDX // 16 + 1         # 34: per-group idx-table cols, padded so each
                             # group's idx base stays 4-byte aligned
SER = T + 1                  # series cols per state (col 0 == t=-1)
KW = [64.0, 64.0, 64.0 * float(np.exp(0.25)), 64.0 * float(np.exp(0.25))]

_CACHE = {}


# ----------------------------------------------------------------------------
# host-side tables
# ----------------------------------------------------------------------------

def _host_tables(y_true):
    """Wrapped gather tables per core: [NCORES, 128, NGRP*NIDX/16] int16.

    pt tiles hold [(a t) part, 16 e, 128 c] — partition p = a*64 + t where
    a picks the 16-example subgroup.  Partitions 0..63 gather subgroup a=0
    (examples g*32+0..15), partitions 64..127 subgroup a=1.  Per subgroup:
    flat[j*16 + e] = e*C + cls(e, j); j=0 is blank, j>=1 label j-1.
    """
    lab = np.asarray(y_true).astype(np.int32)
    cls = np.full((B_TOT, NST), C - 1, np.int32)
    cls[:, 1:] = lab
    tables = np.zeros((NCORES, NGRP, 128, GST), np.int16)
    for core in range(NCORES):
        for g in range(NGRP):
            for a in range(2):
                b0 = core * B + g * EG + a * 16
                flat = (np.arange(16)[None, :] * C
                        + cls[b0:b0 + 16].T).reshape(-1)
                wrapped = flat.reshape(NIDX // 16, 16).T   # [16, NIDX/16]
                tables[core, g, a * 64:(a + 1) * 64, :NIDX // 16] = np.tile(
                    wrapped, (4, 1)).astype(np.int16)
    return np.ascontiguousarray(tables.transpose(0, 2, 1, 3).reshape(
        NCORES, 128, -1))


def _host_mask2(y_true):
    """m2[b, j] for odd state 2j-1: 1 if the s-2 skip is allowed. [B_TOT, NST]."""
    lab = np.asarray(y_true).astype(np.int32)
    m2 = np.zeros((B_TOT, NST), np.float32)
    m2[:, 2:] = (lab[:, 1:] != lab[:, :-1]).astype(np.float32)
    return m2


# ----------------------------------------------------------------------------
# device kernel
# ----------------------------------------------------------------------------

def _build_module():
    import concourse.bacc as bacc
    import concourse.mybir as mybir
    import concourse.tile as tile
    from concourse import library_config
    from concourse.tile_rust import add_dep_helper

    dt = mybir.dt
    AF = mybir.ActivationFunctionType
    OP = mybir.AluOpType

    nc = bacc.Bacc("TRN2", target_bir_lowering=False, debug=False,
                   enable_asserts=False, num_devices=NCORES)

    # y_pred pre-shuffled on host: [(a t), (w g e c)] so each (w, g) DMA
    # is one fully-contiguous 8KB-per-partition transfer on 128 partitions.
    yp = nc.dram_tensor("yp2", [128, NWIN * NGRP * 16 * C], dt.float32,
                        kind="ExternalInput")
    gtab = nc.dram_tensor("gtab", [128, NGRP * GST], dt.int16,
                          kind="ExternalInput")
    m2_in = nc.dram_tensor("m2", [B, NST], dt.float32, kind="ExternalInput")
    ident_in = nc.dram_tensor("ident", [64, 64], dt.float32,
                              kind="ExternalInput")
    fin_out = nc.dram_tensor("fin", [B, 1], dt.float32, kind="ExternalOutput")

    with tile.TileContext(nc) as tc:
        with (
            tc.tile_pool(name="const", bufs=1) as cpool,
            tc.tile_pool(name="pin", bufs=4) as ppool,
            tc.tile_pool(name="eg", bufs=2) as gpool,
            tc.tile_pool(name="ybuf", bufs=8) as ypool,
            tc.tile_pool(name="small", bufs=1) as spool,
            tc.tile_pool(name="tp", bufs=8, space="PSUM") as tpool,
        ):
            ident_sb = cpool.tile([64, 64], dt.float32, name="ident_sb")
            nc.sync.dma_start(ident_sb, ident_in[:, :])
            gtab_sb = cpool.tile([128, NGRP * GST], dt.int16,
                                 name="gtab_sb")
            nc.sync.dma_start(gtab_sb, gtab[:, :])
            m2_sb = cpool.tile([B, NST], dt.float32, name="m2_sb")
            nc.sync.dma_start(m2_sb, m2_in[:, :])

            lib_inst = nc.gpsimd.load_library(library_config.ap_gather)

            # alpha_hat series: [128, S, SER] fp32; col 0 = t=-1 (zeros)
            series = spool.tile([B, S * SER], dt.float32, name="series")
            ser_v = series.rearrange("p (s t) -> p s t", t=SER)
            nc.vector.memset(ser_v[:, :, 0], 0.0)

            zeros_f = spool.tile([B, TW], dt.float32, name="zeros_f")
            nc.vector.memset(zeros_f, 0.0)

            epsb = []
            for w in range(NWIN):
                if w > 0 and KW[w] == KW[w - 1]:
                    epsb.append(epsb[-1])
                    continue
                eb = spool.tile([128, 1], dt.float32, name=f"epsb{w}")
                nc.vector.memset(eb, KW[w] * EPS)
                epsb.append(eb)

            fin = spool.tile([B, 1], dt.float32, name="fin")

            # emission rows per window: [128 ex, NST*TW] fp32
            ecomb = []
            for w in range(NWIN):
                e_t = spool.tile([B, NST * TW], dt.float32, name=f"ecomb{w}")
                ecomb.append(e_t)

            def prep_window(w):
                """DMA + gather + reorder + transpose + scaled copy."""
                egath = gpool.tile([128, NGRP * NIDX], dt.float32, tag="eg",
                                   name=f"egath{w}")
                est = gpool.tile([64, NST * B], dt.float32, tag="est",
                                 name=f"est{w}")
                esv = est.rearrange("p (j g a e) -> p j g a e",
                                    g=NGRP, a=2, j=NST)
                for g in range(NGRP):
                    # [(a t) part, 16 e, 128 c]: a = 16-example subgroup
                    ptile = ppool.tile([128, 16 * C], dt.float32, tag="pt",
                                       name=f"pt{w}_{g}")
                    blk = (w * NGRP + g) * 16 * C
                    nc.sync.dma_start(ptile, yp[:, blk:blk + 16 * C])
                    gi = nc.gpsimd.ap_gather(
                        egath[:, g * NIDX:(g + 1) * NIDX], ptile,
                        gtab_sb[:, g * GST:g * GST + NIDX // 16],
                        channels=128, num_elems=16 * C, d=1, num_idxs=NIDX)
                    add_dep_helper(lib_inst.ins, gi.ins, sync=False,
                                   reason="library before gather")
                    # reorder both subgroup halves into example-major est
                    egv = egath[:, g * NIDX:(g + 1) * NIDX] \
                        .rearrange("p (j e) -> p j e", j=NST)
                    for a in range(2):
                        nc.scalar.activation(
                            esv[:, :, g, a, :],
                            egv[a * 64:(a + 1) * 64, :, :], AF.Copy)
                # PE transpose per state, ACT copy psum->E with
                # (x+eps)*K fused via scale/bias.
                ecv = ecomb[w]
                for j0 in range(0, NST, 4):
                    ns = min(4, NST - j0)
                    tp = tpool.tile([128, 4 * TW], dt.float32, tag="tp",
                                    name=f"tp{w}_{j0}")
                    for k in range(ns):
                        nc.tensor.transpose(
                            tp[:, k * TW:(k + 1) * TW],
                            est[:, (j0 + k) * B:(j0 + k + 1) * B], ident_sb)
                    nc.scalar.activation(
                        ecv[:, j0 * TW:(j0 + ns) * TW],
                        tp[:, 0:ns * TW], AF.Identity,
                        bias=epsb[w], scale=KW[w])

            def sweep_window(w, eng):
                """Run the s-sweep scans for window w on engine `eng`."""
                t0 = w * TW
                ecv = ecomb[w].rearrange("p (j t) -> p j t", t=TW)
                for s in range(S):
                    out_ap = ser_v[:, s, t0 + 1:t0 + 1 + TW]
                    if w == 0:
                        init = 1.0 if s <= 1 else 0.0
                    else:
                        init = ser_v[:, s, t0:t0 + 1]
                    if s == 0:
                        d0 = zeros_f
                    elif s % 2 == 0 or s == 1:
                        d0 = ser_v[:, s - 1, t0:t0 + TW]
                    else:
                        j = (s + 1) // 2
                        yb = ypool.tile([B, TW], dt.float32, tag="yb",
                                        name=f"yb{w}_{s}")
                        eng.scalar_tensor_tensor(
                            yb, ser_v[:, s - 2, t0:t0 + TW],
                            m2_sb[:, j:j + 1], ser_v[:, s - 1, t0:t0 + TW],
                            op0=OP.mult, op1=OP.add)
                        d0 = yb
                    ej = 0 if s % 2 == 0 else (s + 1) // 2
                    eng.tensor_tensor_scan(
                        out_ap, d0, ecv[:, ej, :], init,
                        op0=OP.add, op1=OP.mult)

            for w in range(NWIN):
                prep_window(w)
            for w in range(NWIN):
                sweep_window(w, nc.vector)

            nc.vector.tensor_add(fin, ser_v[:, S - 2, T:T + 1],
                                 ser_v[:, S - 1, T:T + 1])
            nc.sync.dma_start(fin_out[:, :], fin)

    nc.compile()
    return nc


def _get_module():
    if "nc" not in _CACHE:
        _CACHE["nc"] = _build_module()
    return _CACHE["nc"]


# ----------------------------------------------------------------------------
# entry point
# ----------------------------------------------------------------------------

def _feeds(y_true, y_pred):
    y_pred = np.asarray(y_pred, dtype=np.float32)
    tables = _host_tables(y_true)
    m2 = _host_mask2(y_true)
    ident = np.eye(64, dtype=np.float32)
    maps = []
    for core in range(NCORES):
        ypc = y_pred[core * B:(core + 1) * B]
        # [(a tl), (w g e c)]: partition p = a*64 + tl
        yp2 = np.ascontiguousarray(
            ypc.reshape(NGRP, 2, 16, NWIN, TW, C)
            .transpose(1, 4, 3, 0, 2, 5).reshape(128, -1))
        maps.append({
            "yp2": yp2,
            "gtab": tables[core],
            "m2": m2[core * B:(core + 1) * B],
            "ident": ident,
        })
    return maps


def _run(y_true, y_pred, trace=False):
    from concourse.bass_utils import run_bass_kernel_spmd
    nc = _get_module()
    return run_bass_kernel_spmd(nc, _feeds(y_true, y_pred),
                                core_ids=list(range(NCORES)), trace=trace)


def kernel(y_true, y_pred):
    res = _run(y_true, y_pred)
    lnk = TW * float(np.sum(np.log(KW)))
    out = np.zeros(B_TOT, np.float64)
    for i in range(NCORES):
        fin = res.results[i]["fin"].reshape(B).astype(np.float64)
        out[i * B:(i + 1) * B] = lnk - np.log(fin)
    return out.astype(np.float32)[:, None]


def profile_once(y_true, y_pred):
    res = _run(y_true, y_pred, trace=True)
    return res.exec_time_ns


if __name__ == "__main__":
    rng = np.random.default_rng(0)
    yt = rng.integers(0, 126, size=(B_TOT, L)).astype(np.int64)
    logits = rng.standard_normal((B_TOT, T, C)).astype(np.float32)
    ex = np.exp(logits - logits.max(-1, keepdims=True))
    ypred = (ex / ex.sum(-1, keepdims=True)).astype(np.float32)
    out = kernel(yt, ypred)
    print("out", out.shape, out[:4, 0])
